# revision 6
# baseline (speedup 1.0000x reference)
"""Trainium2 Bass kernel for nn_CNN_PHMM_VAE loss (profile-HMM forward + VAE KLD).

Data parallel over 8 NeuronCores (64 examples per core). PHMM forward in
probability space with periodic rescaling; emission lookups are shipped as a
host-gathered bf16 table ee[b,l,k] = A1[b,k+1]*exp(emission[b,k,s[b,l]]),
removing the two scalar_tensor_tensor Horner ops from the per-step loop.

Per step (all plain 2D ops, baseline-shaped):
  beta[1:] = U o mu      ;  delta = scan(V, beta)  ;  t = y + delta
  mu'      = ee_l o t    ;  r12   = [G1-G2|G2] o X ;  ya = r1 + r2
  y'       = mu' + ya

PHASED TIMING BUILD: after the real loop (phase A), scratch phases B/C/E
re-run 64-step loop variants for cadence measurement via the trace.
"""
import numpy as np

B, L, K, E = 512, 256, 64, 16
K1 = K + 1
N_CORES = 8
BPC = B // N_CORES
R = 16
LOGACC0 = -60.0
NEG = -100.0
M2M, M2I, M2D, I2M, I2I, D2M, D2D = 0, 1, 2, 3, 4, 5, 6

# --- small f32 table layout -------------------------------------------------
OFF_X0 = 0             # 132: initial [mu | pad | y | pad]
OFF_W3 = 132           # 198: rows of 66: [U1 | G1-G2 | G2]  (U1[i] = U[i+1])
OFF_UB = 330           # 65:  U (for baseline-shaped beta mul)
OFF_V = 396            # 65
OFF_GG = 462           # 132: [G1-G2 | G2] aligned to X layout
OFF_A1C0 = 594
OFF_A3C0 = 595
OFF_SIG0 = 596
OFF_MUS = 598          # 16
OFF_LV = 614           # 16
TBL_W = 630

XW = 132   # state: mu 0..64, pad, y 66..130, pad
YO = 66

PHASES = True   # include scratch timing phases B/C/E

_CACHED = {}


def _host_tables(batch_input, transition_probs, emission_probs, mus, logvars):
    import ml_dtypes

    a = np.asarray(transition_probs, np.float64)
    Earr = np.exp(np.asarray(emission_probs, np.float64))
    s = np.asarray(batch_input)
    A1 = np.exp(a[:, :, M2M])
    A2 = np.exp(a[:, :, I2M])
    A3 = np.exp(a[:, :, D2M])
    B1 = 0.25 * np.exp(a[:, :, M2I])
    B2 = 0.25 * np.exp(a[:, :, I2I])
    C1 = np.exp(a[:, :, M2D])
    C2 = np.exp(a[:, :, D2D])

    U = np.zeros((B, K1)); V = np.zeros((B, K1))
    U[:, 1:] = A3[:, 1:] * C1[:, :-1] / A1[:, :-1]
    V[:, 1:] = A3[:, 1:] * C2[:, :-1] / A3[:, :-1]
    G1 = A2 * B1 / A1
    G2 = B2

    Etil = A1[:, 1:, None] * Earr
    ee = Etil[np.arange(B)[:, None, None], np.arange(K)[None, None, :],
              s[:, :, None]]
    ee_bf = np.asarray(ee, ml_dtypes.bfloat16).reshape(B, L * K)

    sig0 = np.exp(NEG - LOGACC0)
    e0 = np.exp(-LOGACC0)
    mu0 = np.empty((B, K1)); iot0 = np.empty((B, K1))
    mu0[:, 0] = A1[:, 0] * e0
    mu0[:, 1:] = A1[:, 1:] * sig0
    iot0[:, :] = A2 * sig0

    tbl = np.zeros((B, TBL_W), np.float32)
    tbl[:, OFF_X0:OFF_X0 + K1] = mu0
    tbl[:, OFF_X0 + YO:OFF_X0 + YO + K1] = mu0 + iot0
    tbl[:, OFF_W3:OFF_W3 + K] = U[:, 1:]
    tbl[:, OFF_W3 + 66:OFF_W3 + 66 + K1] = G1 - G2
    tbl[:, OFF_W3 + 132:OFF_W3 + 132 + K1] = G2
    tbl[:, OFF_UB:OFF_UB + K1] = U
    tbl[:, OFF_V:OFF_V + K1] = V
    tbl[:, OFF_GG:OFF_GG + K1] = G1 - G2
    tbl[:, OFF_GG + YO:OFF_GG + YO + K1] = G2
    tbl[:, OFF_A1C0] = A1[:, 0]
    tbl[:, OFF_A3C0] = A3[:, 0]
    tbl[:, OFF_SIG0] = sig0
    tbl[:, OFF_MUS:OFF_MUS + E] = np.asarray(mus, np.float32)
    tbl[:, OFF_LV:OFF_LV + E] = np.asarray(logvars, np.float32)
    return tbl, ee_bf


def _build_bass():
    import concourse.tile as tile
    from concourse import bacc, mybir
    from concourse.ap import AP
    from contextlib import ExitStack

    f32 = mybir.dt.float32
    bf = mybir.dt.bfloat16
    mult = mybir.AluOpType.mult
    add = mybir.AluOpType.add
    mx_op = mybir.AluOpType.max
    AF = mybir.ActivationFunctionType

    nc = bacc.Bacc("TRN2", target_bir_lowering=False, debug=False,
                   num_devices=N_CORES)
    tbl_d = nc.dram_tensor("tbl", [BPC, TBL_W], f32, kind="ExternalInput").ap()
    ee_d = nc.dram_tensor("ee", [BPC, L * K], bf, kind="ExternalInput").ap()
    out_d = nc.dram_tensor("loss", [BPC, 1], f32, kind="ExternalOutput").ap()

    def strided(ap, dims):
        return AP(ap.tensor, ap.offset, [list(ap.ap[0])] + dims)

    with tile.TileContext(nc) as tc, ExitStack() as ctx:
        ctx.enter_context(nc.allow_low_precision(
            reason="bf16 DP state validated to ~2e-4 per-example on the loss"))
        pool = ctx.enter_context(tc.tile_pool(name="p", bufs=1))

        TBL = pool.tile([BPC, TBL_W], f32, tag="TBL", name="TBL")
        EEt = pool.tile([BPC, L * K], bf, tag="EE", name="EE")
        nc.sync.dma_start(TBL[:, :], tbl_d[:, :])
        NCH = 4
        CW = L * K // NCH
        for c in range(NCH):
            nc.sync.dma_start(EEt[:, c * CW:(c + 1) * CW],
                              ee_d[:, c * CW:(c + 1) * CW])

        def tb(off, n):
            return TBL[:, off:off + n]

        v = nc.vector

        # KLD
        ev = pool.tile([BPC, E], f32, tag="ev", name="ev")
        sq = pool.tile([BPC, E], f32, tag="sq", name="sq")
        w1 = pool.tile([BPC, E], f32, tag="w1", name="w1")
        w2 = pool.tile([BPC, E], f32, tag="w2", name="w2")
        red = pool.tile([BPC, 1], f32, tag="red", name="red")
        kld = pool.tile([BPC, 1], f32, tag="kld", name="kld")
        nc.scalar.activation(ev[:, :], tb(OFF_LV, E), AF.Exp)
        nc.scalar.activation(sq[:, :], tb(OFF_MUS, E), AF.Square)
        v.tensor_sub(w1[:, :], tb(OFF_LV, E), sq[:, :])
        v.tensor_sub(w2[:, :], w1[:, :], ev[:, :])
        v.tensor_reduce(red[:, :], w2[:, :], mybir.AxisListType.X, add)
        v.tensor_scalar(kld[:, :], red[:, :], -0.5, -float(E) / 2.0, mult, add)

        # ---- phase A: real computation, baseline-shaped 7-op loop ----------
        x_pp = [pool.tile([BPC, XW], bf, tag="x_a", name="x_a"),
                pool.tile([BPC, XW], bf, tag="x_b", name="x_b")]
        Ub = pool.tile([BPC, K1], bf, tag="Ub", name="Ub")
        Vb = pool.tile([BPC, K1], bf, tag="Vb", name="Vb")
        GGb = pool.tile([BPC, XW], bf, tag="GGb", name="GGb")
        beta = pool.tile([BPC, K1], bf, tag="beta", name="beta")
        delta = pool.tile([BPC, K1], bf, tag="delta", name="delta")
        t = pool.tile([BPC, K], bf, tag="t", name="t")
        r12 = pool.tile([BPC, XW], bf, tag="r12", name="r12")
        ya = pool.tile([BPC, K1 + 1], bf, tag="ya", name="ya")
        sig = pool.tile([BPC, 1], f32, tag="sig", name="sig")
        rmxb = pool.tile([BPC, 1], bf, tag="rmxb", name="rmxb")
        mxt = pool.tile([BPC, 1], f32, tag="mxt", name="mxt")
        NRS = L // R - 1
        rhist = pool.tile([BPC, NRS], f32, tag="rhist", name="rhist")

        v.memset(x_pp[0][:, :], 0.0)
        v.memset(x_pp[1][:, :], 0.0)
        v.memset(r12[:, :], 0.0)
        v.tensor_copy(Ub[:, :], tb(OFF_UB, K1))
        v.tensor_copy(Vb[:, :], tb(OFF_V, K1))
        v.memset(GGb[:, :], 0.0)
        v.tensor_copy(GGb[:, 0:K1], tb(OFF_GG, K1))
        v.tensor_copy(GGb[:, YO:YO + K1], tb(OFF_GG + YO, K1))
        v.tensor_copy(x_pp[0][:, 0:K1], tb(OFF_X0, K1))
        v.tensor_copy(x_pp[0][:, YO:YO + K1], tb(OFF_X0 + YO, K1))
        v.tensor_copy(sig[:, :], tb(OFF_SIG0, 1))
        v.tensor_mul(beta[:, 0:1], tb(OFF_A3C0, 1), sig[:, :])
        v.tensor_mul(x_pp[1][:, 0:1], tb(OFF_A1C0, 1), sig[:, :])

        def dp_step(l):
            X, Xn = x_pp[l % 2], x_pp[(l + 1) % 2]
            v.tensor_mul(beta[:, 1:K1], Ub[:, 1:K1], X[:, 0:K])
            v.tensor_tensor_scan(delta[:, :], Vb[:, :], beta[:, :], 0.0,
                                 mult, add)
            v.tensor_add(t[:, :], X[:, YO:YO + K], delta[:, 0:K])
            v.tensor_mul(Xn[:, 1:K1], EEt[:, l * K:(l + 1) * K], t[:, :])
            v.tensor_mul(r12[:, :], GGb[:, :], X[:, :])
            v.tensor_add(ya[:, :], r12[:, 0:K1 + 1], r12[:, YO:YO + K1 + 1])
            v.tensor_add(Xn[:, YO:YO + K1 + 1], Xn[:, 0:K1 + 1], ya[:, :])

        def rescale(i, l):
            cur = (l + 1) % 2
            Xc = x_pp[cur]
            x_stale = x_pp[1 - cur]
            rmx = rhist[:, i:i + 1]
            v.tensor_reduce(mxt[:, :], Xc[:, YO:YO + K1], mybir.AxisListType.X,
                            mx_op)
            v.reciprocal(rmxb[:, :], mxt[:, :])
            v.tensor_copy(rmx, rmxb[:, :])
            v.tensor_scalar_mul(Xc[:, :], Xc[:, :], rmx)
            v.tensor_scalar_mul(sig[:, :], sig[:, :], rmx)
            v.tensor_mul(beta[:, 0:1], tb(OFF_A3C0, 1), sig[:, :])
            v.tensor_mul(x_stale[:, 0:1], tb(OFF_A1C0, 1), sig[:, :])

        for l in range(L):
            dp_step(l)
            if l == 0:
                v.tensor_mul(x_pp[0][:, 0:1], tb(OFF_A1C0, 1), sig[:, :])
            if (l + 1) % R == 0 and (l + 1) < L:
                rescale((l + 1) // R - 1, l)

        Xf = x_pp[L % 2]
        tf = pool.tile([BPC, K1], f32, tag="tf", name="tf")
        lnp = pool.tile([BPC, 1], f32, tag="lnp", name="lnp")
        lnr = pool.tile([BPC, NRS], f32, tag="lnr", name="lnr")
        sumlr = pool.tile([BPC, 1], f32, tag="sumlr", name="sumlr")
        lacc = pool.tile([BPC, 1], f32, tag="lacc", name="lacc")
        nv = pool.tile([BPC, 1], f32, tag="nv", name="nv")
        v.tensor_mul(beta[:, 1:K1], Ub[:, 1:K1], Xf[:, 0:K])
        v.tensor_tensor_scan(delta[:, :], Vb[:, :], beta[:, :], 0.0, mult, add)
        v.tensor_add(tf[:, :], Xf[:, YO:YO + K1], delta[:, :])
        nc.scalar.activation(lnp[:, :], tf[:, K:K1], AF.Ln)
        nc.scalar.activation(lnr[:, :], rhist[:, :], AF.Ln)
        v.tensor_reduce(sumlr[:, :], lnr[:, :], mybir.AxisListType.X, add)
        v.tensor_scalar(lacc[:, :], sumlr[:, :], -1.0, LOGACC0, mult, add)
        v.tensor_add(nv[:, :], lnp[:, :], lacc[:, :])
        loss_t = pool.tile([BPC, 1], f32, tag="loss_t", name="loss_t")
        v.tensor_sub(loss_t[:, :], kld[:, :], nv[:, :])
        nc.sync.dma_start(out_d[:, :], loss_t[:, :])

        if PHASES:
            NS = 64  # scratch steps per phase
            # ---- phase B: same 7-op loop, EE from engine-copied buffer -----
            EE2 = pool.tile([BPC, NS * K], bf, tag="EE2", name="EE2")
            v.tensor_copy(EE2[:, :], EEt[:, 0:NS * K])
            x2 = [pool.tile([BPC, XW], bf, tag="x2a", name="x2a"),
                  pool.tile([BPC, XW], bf, tag="x2b", name="x2b")]
            # serialize phase B after A by seeding from A's final state
            v.tensor_copy(x2[0][:, :], Xf[:, :])
            v.tensor_copy(x2[1][:, :], Xf[:, :])
            for l in range(NS):
                X, Xn = x2[l % 2], x2[(l + 1) % 2]
                v.tensor_mul(beta[:, 1:K1], Ub[:, 1:K1], X[:, 0:K])
                v.tensor_tensor_scan(delta[:, :], Vb[:, :], beta[:, :], 0.0,
                                     mult, add)
                v.tensor_add(t[:, :], X[:, YO:YO + K], delta[:, 0:K])
                v.tensor_mul(Xn[:, 1:K1], EE2[:, l * K:(l + 1) * K], t[:, :])
                v.tensor_mul(r12[:, :], GGb[:, :], X[:, :])
                v.tensor_add(ya[:, :], r12[:, 0:K1 + 1], r12[:, YO:YO + K1 + 1])
                v.tensor_add(Xn[:, YO:YO + K1 + 1], Xn[:, 0:K1 + 1], ya[:, :])

            # ---- phase C: fused 4-row Z-mul via native rearranged APs ------
            W4b = pool.tile([BPC, 264], bf, tag="W4b", name="W4b")
            v.memset(W4b[:, :], 0.0)
            v.tensor_copy(W4b[:, 0:198], tb(OFF_W3, 198))
            ZC = pool.tile([BPC, 266], bf, tag="ZC", name="ZC")
            v.memset(ZC[:, :], 0.0)
            x3 = [pool.tile([BPC, XW], bf, tag="x3a", name="x3a"),
                  pool.tile([BPC, XW], bf, tag="x3b", name="x3b")]
            v.tensor_copy(x3[0][:, :], x2[0][:, :])   # serialize after B
            v.tensor_copy(x3[1][:, :], x2[0][:, :])
            v.tensor_mul(ZC[:, 0:1], tb(OFF_A3C0, 1), sig[:, :])
            w4r = W4b[:, 0:264].rearrange("p (a c b) -> p a c b", a=2, c=2)
            zcr = ZC[:, 1:265].rearrange("p (a c b) -> p a c b", a=2, c=2)
            for l in range(NS):
                X, Xn = x3[l % 2], x3[(l + 1) % 2]
                xsrc = (X[:, 0:132].rearrange("p (a b) -> p a b", a=2)
                        .unsqueeze(2).broadcast_to((BPC, 2, 2, 66)))
                v.tensor_mul(zcr, w4r, xsrc)
                v.tensor_tensor_scan(delta[:, :], Vb[:, :], ZC[:, 0:K1], 0.0,
                                     mult, add)
                v.tensor_add(t[:, :], X[:, YO:YO + K], delta[:, 0:K])
                v.tensor_mul(Xn[:, 1:K1], EE2[:, l * K:(l + 1) * K], t[:, :])
                v.tensor_add(ya[:, 0:K1], ZC[:, 67:67 + K1], ZC[:, 133:133 + K1])
                v.tensor_add(Xn[:, YO:YO + K1], Xn[:, 0:K1], ya[:, 0:K1])

            # ---- phase E: v2-style raw strided APs, 6-op loop --------------
            ZT = pool.tile([BPC, 400], bf, tag="ZT", name="ZT")
            v.memset(ZT[:, :], 0.0)
            x4 = [pool.tile([BPC, 200], bf, tag="x4a", name="x4a"),
                  pool.tile([BPC, 200], bf, tag="x4b", name="x4b")]
            v.memset(x4[0][:, :], 0.0)
            v.memset(x4[1][:, :], 0.0)
            v.tensor_copy(x4[0][:, 0:132], x3[0][:, :])   # serialize after C
            W3c = pool.tile([BPC, 200], bf, tag="W3c", name="W3c")
            v.memset(W3c[:, :], 0.0)
            v.tensor_copy(W3c[:, 0:198], tb(OFF_W3, 198))
            for l in range(NS):
                X, Xn = x4[l % 2], x4[(l + 1) % 2]
                Z = ZT[:, (l % 2) * 200:(l % 2) * 200 + 200]
                v.tensor_mul(strided(Z[:, 1:2], [[66, 3], [1, K1]]),
                             strided(W3c[:, 0:1], [[66, 3], [1, K1]]),
                             strided(X[:, 0:1], [[66, 3], [1, K1]]))
                v.tensor_tensor_scan(delta[:, :], Vb[:, :], Z[:, 0:K1], 0.0,
                                     mult, add)
                v.tensor_add(t[:, :], X[:, 132:132 + K], delta[:, 0:K])
                v.tensor_mul(strided(Xn[:, 1:2], [[66, 2], [1, K]]),
                             EEt[:, l * K:(l + 1) * K].unsqueeze(1)
                                .broadcast_to((BPC, 2, K)),
                             t[:, :].unsqueeze(1).broadcast_to((BPC, 2, K)))
                v.tensor_add(ya[:, 0:K1], Z[:, 67:67 + K1], Z[:, 133:133 + K1])
                v.tensor_add(Xn[:, 132:132 + K1], Xn[:, 0:K1], ya[:, 0:K1])

    nc.compile()
    return nc


def _get_nc():
    if "nc" not in _CACHED:
        _CACHED["nc"] = _build_bass()
    return _CACHED["nc"]


def kernel(batch_input, transition_probs, emission_probs, mus, logvars):
    from concourse.bass_utils import run_bass_kernel_spmd

    tbl, ee = _host_tables(batch_input, transition_probs, emission_probs,
                           mus, logvars)
    nc = _get_nc()
    in_maps = [{"tbl": tbl[c * BPC:(c + 1) * BPC],
                "ee": ee[c * BPC:(c + 1) * BPC]} for c in range(N_CORES)]
    res = run_bass_kernel_spmd(nc, in_maps, list(range(N_CORES)))
    losses = np.concatenate([np.asarray(r["loss"])[:, 0] for r in res.results])
    return np.float32(np.mean(losses.astype(np.float64)))
